# revision 9
# baseline (speedup 1.0000x reference)
"""Trainium2 Bass kernel for nn_Attention_5420248728069.

Data-parallel over 8 NeuronCores on v_code rows; obs_code + weights
replicated; no collectives.

Math (exact refactoring of the reference):
    A   = Wq.T @ Wk                      # [E, E]
    S   = (v @ A) @ obs.T ;  s_self = rowsum((v@A) * v)
    y   = ((w0*v + expS @ obs) @ Wv.T) / Z + v    # unnormalized softmax
    out = LayerNorm(y) * gamma + beta

Token permutation: internal index n~ = 128*c + p <-> row n = 8*p + c
(partition-major DMA -> contiguous multi-KB descriptors; the output store
uses the same mapping so rows land canonically).

fp8 scaling (all folded, zero extra ops): A stored x4, WvT stored x8,
uT accumulated /32, recipZ = (1/(UTSCALE*WVSCALE))/Z; exp scale
1/(4*TEMP).  No logit shift: exp(logits/TEMP) spans ~e^+-5 which fits
fp8e4 (max 448) with much better resolution than a shifted-subnormal
encoding.

Schedule (per core):
 - DMA in 0.5MB pieces, two HWDGE queues, interleaved so obs chunk pairs
   arrive roughly in consumption order while Wq/Wk/v land early.
 - PE warmup transposes keep the HAM clock at 2.4GHz through the lead-in.
 - obs f32->f8 casts on GpSimd (otherwise idle).
 - obs transposes software-pipelined inside block-0's m-loop, two
   iterations ahead of consumption.
 - m-loop per n~-block over 32 obs chunks: scores (fp8 DR matmul) ->
   exp (ScalarE) -> Z row-sum + uT accumulation (fp8 DR matmuls).  The
   Z/uT matmuls of iteration t are issued during iteration t+1 so they
   never wait on exp latency.
 - second-half vT/vAT, self-score row, w0 broadcast (gpsimd
   partition_broadcast; no DRAM roundtrip), WvT transposes are slotted
   into m-loop iterations where their DMA inputs have landed.
 - Z -> per-token recipZ via rank-1 matmuls (on-chip transpose).
 - epilogue per 128-token chunk (uT.T@WvT, *recipZ + v residual,
   LayerNorm: bn_stats + pow(-0.5) on DVE, gamma/beta on GpSimd, store
   on sync queue), interleaved into the NEXT block's m-loop; the last
   block is 128 tokens so the tail is one chunk.
"""

import numpy as np

N_GLOBAL = 8192
M = 4096
E = 512
CORES = 8
NLOC = N_GLOBAL // CORES  # 1024
TEMPERATURE = 22.627416997969522  # sqrt(E)
EPS = 1e-6
P = 128

NCH = NLOC // P  # 8 token chunks
MCH = M // P  # 32 obs chunks

# n~ blocks (multiples of 128; uT psum = 4*size*4B <= 8KB/partition)
BLOCKS = [512, 384, 128]
assert sum(BLOCKS) == NLOC

ASCALE = 4.0  # A stored x4 (keeps fp8e4 out of subnormals)
WVSCALE = 8.0  # WvT stored x8
SHIFT = 2.0  # softmax logit shift: exp stays under fp8e4 max (448)
UTSCALE = 1.0 / 8.0  # uT accumulation scale
EXPSCALE = 1.0 / (ASCALE * TEMPERATURE)
RZSCALE = 1.0 / (UTSCALE * WVSCALE)  # recipZ = RZSCALE / Z

_CACHED_NC = None


def _build():
    from contextlib import ExitStack

    import concourse.bass as bass
    import concourse.tile as tile
    from concourse import bacc, mybir
    from concourse.masks import make_identity

    f32 = mybir.dt.float32
    bf16 = mybir.dt.bfloat16
    f8 = mybir.dt.float8e4
    DR = mybir.MatmulPerfMode.DoubleRow
    AF = mybir.ActivationFunctionType
    ALU = mybir.AluOpType

    nc = bacc.Bacc("TRN2", target_bir_lowering=False, debug=False)

    v_d = nc.dram_tensor("v_code", [NLOC, E], f32, kind="ExternalInput")
    obs_d = nc.dram_tensor("obs_code", [M, E], f32, kind="ExternalInput")
    wq_d = nc.dram_tensor("Wq", [E, E], f32, kind="ExternalInput")
    wk_d = nc.dram_tensor("Wk", [E, E], f32, kind="ExternalInput")
    wv_d = nc.dram_tensor("Wv", [E, E], f32, kind="ExternalInput")
    gamma_d = nc.dram_tensor("gamma", [E], f32, kind="ExternalInput")
    beta_d = nc.dram_tensor("beta", [E], f32, kind="ExternalInput")
    out_d = nc.dram_tensor("out", [NLOC, E], f32, kind="ExternalOutput")
    import os
    dbg = os.environ.get("KDEBUG") == "1"
    if dbg:
        dbg_t = {}
        for nm, shp, dt in [
            ("dbg_obs8", [P, MCH, E], f8), ("dbg_obsT", [P, 4, M], f8),
            ("dbg_vT", [P, 4, NLOC], f8), ("dbg_vAT", [P, 4, NLOC], f8),
            ("dbg_A", [P, 4, E], f8), ("dbg_WvT", [P, 4, E], f8),
            ("dbg_w0row", [1, NLOC], f32), ("dbg_w0bc", [P, NLOC], f32),
            ("dbg_uT0", [P, 4, 512], f8), ("dbg_zr0", [1, 512], f32),
            ("dbg_rz0", [P, 4], f32),
        ]:
            dbg_t[nm] = nc.dram_tensor(nm, shp, dt, kind="ExternalOutput")

    def bcast_ap(ap_1row, parts=P):
        dims = [list(d) for d in ap_1row.ap]
        if len(dims) > 1 and dims[0][1] == 1:
            dims = dims[1:]
        return bass.AP(
            tensor=ap_1row.tensor, offset=ap_1row.offset, ap=[[0, parts]] + dims
        )

    with tile.TileContext(nc) as tc, ExitStack() as ctx:
        const = ctx.enter_context(tc.tile_pool(name="const", bufs=1))
        persist = ctx.enter_context(tc.tile_pool(name="persist", bufs=1))
        utp = ctx.enter_context(tc.tile_pool(name="utp", bufs=2))
        expp = ctx.enter_context(tc.tile_pool(name="expp", bufs=4))
        prodp = ctx.enter_context(tc.tile_pool(name="prodp", bufs=4))
        epiy = ctx.enter_context(tc.tile_pool(name="epiy", bufs=2))
        episml = ctx.enter_context(tc.tile_pool(name="episml", bufs=8))
        zsmall = ctx.enter_context(tc.tile_pool(name="zsmall", bufs=4))

        # ---------------- persistent SBUF ----------------
        v_f32 = persist.tile([P, NCH, E], f32, tag="v_f32")
        wq_f = persist.tile([P, 4, E], f32, tag="wq_f")
        wk_f = persist.tile([P, 4, E], f32, tag="wk_f")
        wv_f = persist.tile([P, 4, E], f32, tag="wv_f")
        obs_f32 = persist.tile([P, MCH, E], f32, tag="obs_f32")
        obs_f8 = persist.tile([P, MCH, E], f8, tag="obs_f8")
        obsT = persist.tile([P, 4, M], f8, tag="obsT")
        vT = persist.tile([P, 4, NLOC], f8, tag="vT")
        vAT = persist.tile([P, 4, NLOC], f8, tag="vAT")
        A_sb = persist.tile([P, 4, E], f8, tag="A")
        WvT = persist.tile([P, 4, E], f8, tag="WvT")
        wq_b = persist.tile([P, 4, E], bf16, tag="wq_b")
        wk_b = persist.tile([P, 4, E], bf16, tag="wk_b")
        w0v = persist.tile([P, 4, NLOC], bf16, tag="w0v")
        w0row = persist.tile([1, NLOC], f32, tag="w0row")
        w0bc = persist.tile([P, NLOC], f32, tag="w0bc")

        gamma_b = const.tile([P, E], f32, tag="gamma")
        beta_b = const.tile([P, E], f32, tag="beta")
        identity = const.tile([P, P], f32, tag="ident")
        ident8 = const.tile([P, P], f8, tag="ident8")
        ones_bf = const.tile([P, 1], bf16, tag="ones")
        ones_f8w = const.tile([P, 2, P], f8, tag="ones8w")
        ones_f32 = const.tile([1, 1], f32, tag="ones1")
        eps_t = const.tile([P, 1], f32, tag="eps")
        nshift_t = const.tile([P, 1], f32, tag="nshift")

        make_identity(nc, identity)
        make_identity(nc, ident8)
        nc.vector.memset(ones_bf, 1.0)
        nc.vector.memset(ones_f8w, 1.0)
        nc.vector.memset(ones_f32, 1.0)
        nc.vector.memset(eps_t, EPS)
        nc.vector.memset(nshift_t, -SHIFT)
        nc.gpsimd.dma_start(out=gamma_b, in_=bcast_ap(gamma_d.ap()))
        nc.gpsimd.dma_start(out=beta_b, in_=bcast_ap(beta_d.ap()))

        out_r = out_d.ap().rearrange("(p c) e -> p c e", c=NCH)

        # ---------------- DMA issue (queue order = service order) -------
        obs_r = obs_d.ap().rearrange("(p c) e -> p c e", c=MCH)
        v_r = v_d.ap().rearrange("(p c) e -> p c e", c=NCH)

        def load_obs_pair(eng, j):  # chunks 2j, 2j+1 (0.5MB)
            eng.dma_start(
                obs_f32[:, 2 * j : 2 * j + 2, :], obs_r[:, 2 * j : 2 * j + 2, :]
            )

        def load_v_quarter(j):  # chunks 2j, 2j+1
            nc.sync.dma_start(
                v_f32[:, 2 * j : 2 * j + 2, :], v_r[:, 2 * j : 2 * j + 2, :]
            )

        # scalar q: o0, Wq, o2, o4, o6, o8, o10, o12, o14, Wv
        load_obs_pair(nc.scalar, 0)
        nc.scalar.dma_start(wq_f, wq_d.ap().rearrange("(p c) e -> p c e", c=4))
        for j in (2, 4, 6, 8, 10, 12, 14):
            load_obs_pair(nc.scalar, j)
        nc.scalar.dma_start(wv_f, wv_d.ap().rearrange("(c p) e -> p c e", p=P))
        # sync q: Wk, v0, o1, v1, o3, v2, o5, v3, o7, o9, o11, o13, o15
        nc.sync.dma_start(wk_f, wk_d.ap().rearrange("(p c) e -> p c e", c=4))
        load_v_quarter(0)
        load_obs_pair(nc.sync, 1)
        load_v_quarter(1)
        load_obs_pair(nc.sync, 3)
        load_v_quarter(2)
        load_obs_pair(nc.sync, 5)
        load_v_quarter(3)
        for j in (7, 9, 11, 13, 15):
            load_obs_pair(nc.sync, j)

        # obs casts f32 -> f8 on GpSimd, in DMA arrival order
        for j in range(16):
            nc.gpsimd.tensor_copy(
                obs_f8[:, 2 * j : 2 * j + 2, :], obs_f32[:, 2 * j : 2 * j + 2, :]
            )

        # ---------------- PSUM pools (8 banks total) ----------------
        ps_s_pool = ctx.enter_context(tc.tile_pool(name="ps_s", bufs=2, space="PSUM"))
        ps_z_pool = ctx.enter_context(tc.tile_pool(name="ps_z", bufs=1, space="PSUM"))
        ps_ut_pool = ctx.enter_context(
            tc.tile_pool(name="ps_ut", bufs=1, space="PSUM")
        )
        ps_sh_pool = ctx.enter_context(
            tc.tile_pool(name="ps_sh", bufs=1, space="PSUM")
        )

        # ---------------- emission helpers ----------------
        def warmup(n):
            w = ps_sh_pool.tile([P, P], f32, tag="sh")
            for _ in range(n):
                nc.tensor.transpose(w, identity, identity)

        def transpose_pair(mc0):
            # obsT[:, :, mc0*P:(mc0+2)*P] <- obs chunks mc0, mc0+1
            pst8 = ps_sh_pool.tile([P, 4, 2, 256], f8, tag="sh")
            for ec in range(4):
                for j in range(2):
                    nc.tensor.transpose(
                        pst8[:, ec, j, 0:256:2],
                        obs_f8[:, mc0 + j, ec * P : (ec + 1) * P],
                        ident8,
                    )
            dst = obsT[:, :, mc0 * P : (mc0 + 2) * P].rearrange(
                "p e (j f) -> p e j f", j=2
            )
            nc.vector.tensor_copy(dst, pst8[:, :, :, 0:256:2])

        def vt_chunk(nk):
            # vT[:, :, nk*P:(nk+1)*P] <- v chunk nk (f32 transposes, f8 drain)
            pst = ps_sh_pool.tile([P, 4, P], f32, tag="sh")
            for ec in range(4):
                nc.tensor.transpose(
                    pst[:, ec, :], v_f32[:, nk, ec * P : (ec + 1) * P], identity
                )
            nc.vector.tensor_copy(vT[:, :, nk * P : (nk + 1) * P], pst)

        def a_matmuls():
            nc.vector.tensor_copy(wq_b, wq_f)
            nc.vector.tensor_copy(wk_b, wk_f)
            for ic in range(4):
                psA = ps_s_pool.tile([P, E], f32, tag="s")
                for kc in range(4):
                    nc.tensor.matmul(
                        psA,
                        lhsT=wq_b[:, kc, ic * P : (ic + 1) * P],
                        rhs=wk_b[:, kc, :],
                        start=(kc == 0),
                        stop=(kc == 3),
                    )
                nc.scalar.mul(A_sb[:, ic, :], psA, ASCALE)

        def vat_quarter(e2, h):
            hs = slice(h * 512, (h + 1) * 512)
            psv = ps_s_pool.tile([P, 512], f32, tag="s")
            for u in range(2):
                nc.tensor.matmul(
                    psv,
                    lhsT=A_sb[:, 2 * u : 2 * u + 2, e2 * P : (e2 + 1) * P],
                    rhs=vT[:, 2 * u : 2 * u + 2, hs],
                    start=(u == 0),
                    stop=(u == 1),
                    perf_mode=DR,
                )
            nc.vector.tensor_copy(vAT[:, e2, hs], psv)

        def prod_half(ec, h):
            hs = slice(h * 512, (h + 1) * 512)
            pr = prodp.tile([P, 512], bf16, tag="prod")
            nc.vector.tensor_mul(pr, vAT[:, ec, hs], vT[:, ec, hs])
            return pr

        def w0_half(h, prods):
            # self-score row -> w0row / w0bc / w0v for n~ half h
            hs = slice(h * 512, (h + 1) * 512)
            ps_sr = ps_sh_pool.tile([1, 512], f32, tag="sh")
            for ec in range(4):
                nc.tensor.matmul(
                    ps_sr,
                    lhsT=ones_bf,
                    rhs=prods[ec],
                    start=(ec == 0),
                    stop=(ec == 3),
                )
            nc.scalar.activation(w0row[:, hs], ps_sr, AF.Exp, bias=nshift_t[0:1, :], scale=EXPSCALE)
            nc.gpsimd.partition_broadcast(w0bc[:, hs], w0row[:, hs])
            for ec in range(4):
                nc.vector.scalar_tensor_tensor(
                    w0v[:, ec, hs],
                    in0=vT[:, ec, hs],
                    scalar=UTSCALE,
                    in1=w0bc[:, hs],
                    op0=ALU.mult,
                    op1=ALU.mult,
                )

        def wvt_quarter(jc):
            pst = ps_sh_pool.tile([P, 4, P], f32, tag="sh")
            for ic in range(4):
                nc.tensor.transpose(
                    pst[:, ic, :], wv_f[:, ic, jc * P : (jc + 1) * P], identity
                )
            nc.scalar.mul(WvT[:, jc, :], pst.rearrange("p a b -> p (a b)"), WVSCALE)

        uts = []
        rzs = []
        nblk = len(BLOCKS)
        starts = [sum(BLOCKS[:i]) for i in range(nblk)]

        def epi_chunk(bi, k):
            q = starts[bi] // P + k
            uTb, rzb = uts[bi], rzs[bi]
            ps_y = ps_sh_pool.tile([P, E], f32, tag="sh")
            for u in range(2):
                nc.tensor.matmul(
                    ps_y,
                    lhsT=uTb[:, 2 * u : 2 * u + 2, k * P : (k + 1) * P],
                    rhs=WvT[:, 2 * u : 2 * u + 2, :],
                    start=(u == 0),
                    stop=(u == 1),
                    perf_mode=DR,
                )
            y2 = epiy.tile([P, E], f32, tag="y2")
            nc.vector.scalar_tensor_tensor(
                y2,
                in0=ps_y,
                scalar=rzb[:, k : k + 1],
                in1=v_f32[:, q, :],
                op0=ALU.mult,
                op1=ALU.add,
            )
            stats = episml.tile([P, 6], f32, tag="stats")
            nc.vector.bn_stats(stats, y2)
            mv = episml.tile([P, 2], f32, tag="mv")
            nc.vector.bn_aggr(mv, stats)
            # rstd = exp(-0.5*ln(var+eps)): Ln and Exp share one ACT
            # table set (natural_log_exp_and_others) -> no table thrash
            # against the main loop's exps, unlike Sqrt.
            lnv = episml.tile([P, 1], f32, tag="lnv")
            nc.scalar.activation(lnv, mv[:, 1:2], AF.Ln, bias=eps_t)
            rstd = episml.tile([P, 1], f32, tag="rstd")
            nc.scalar.activation(rstd, lnv, AF.Exp, scale=-0.5)
            nc.vector.tensor_scalar(
                y2, in0=y2, scalar1=mv[:, 0:1], scalar2=rstd,
                op0=ALU.subtract, op1=ALU.mult,
            )
            nc.gpsimd.tensor_mul(y2, y2, gamma_b)
            nc.gpsimd.tensor_add(y2, y2, beta_b)
            nc.sync.dma_start(out_r[:, q, :], y2)

        # ---------------- phase A ----------------
        warmup(56)
        transpose_pair(0)
        warmup(16)
        vt_chunk(0)
        vt_chunk(1)
        a_matmuls()
        vt_chunk(2)
        vt_chunk(3)
        for e2 in range(4):
            vat_quarter(e2, 0)
        transpose_pair(2)

        # ---------------- main block loop ----------------
        prods_h = {0: [], 1: []}

        for b, (n0, bs) in enumerate(zip(starts, BLOCKS)):
            nsl = slice(n0, n0 + bs)
            nch = bs // P
            ps_uT = ps_ut_pool.tile([P, 4, bs], f32, tag="uT")
            ps_z = ps_z_pool.tile([P, bs], f32, tag="z")

            extras = {t: [] for t in range(16)}
            if b == 0:
                # obs transposes: pair for iteration t+2, i.e. chunks 2t+4/5
                for t in range(14):
                    extras[t].append(lambda mc0=2 * t + 4: transpose_pair(mc0))
                extras[1].append(lambda: prods_h[0].extend(
                    [prod_half(0, 0), prod_half(1, 0)]))
                extras[2].append(lambda: prods_h[0].extend(
                    [prod_half(2, 0), prod_half(3, 0)]))
                extras[3].append(lambda: w0_half(0, prods_h[0]))
                for t in range(3, 7):  # vT second half (v quarters land here)
                    extras[t].append(lambda nk=t + 1: vt_chunk(nk))
                for t in range(7, 11):
                    extras[t].append(lambda e2=t - 7: vat_quarter(e2, 1))
                extras[10].append(lambda: prods_h[1].extend(
                    [prod_half(0, 1), prod_half(1, 1)]))
                extras[11].append(lambda: prods_h[1].extend(
                    [prod_half(2, 1), prod_half(3, 1)]))
                extras[12].append(lambda: w0_half(1, prods_h[1]))
            else:
                if b == 1:
                    for t in range(4):
                        extras[t].append(lambda jc=t: wvt_quarter(jc))
                pch = BLOCKS[b - 1] // P
                slots = [5, 8, 11, 14] if b == 1 else [3, 7, 11]
                for i in range(pch):
                    extras[slots[i]].append(
                        lambda bi=b - 1, k=i: epi_chunk(bi, k)
                    )

            prev_ex = None
            for t in range(16):
                ex2 = expp.tile([P, 2, bs], f8, tag="ex")
                for j in range(2):
                    mc = 2 * t + j
                    ps_s = ps_s_pool.tile([P, bs], f32, tag="s")
                    for u in range(2):
                        nc.tensor.matmul(
                            ps_s,
                            lhsT=obsT[:, 2 * u : 2 * u + 2, mc * P : (mc + 1) * P],
                            rhs=vAT[:, 2 * u : 2 * u + 2, nsl],
                            start=(u == 0),
                            stop=(u == 1),
                            perf_mode=DR,
                        )
                    nc.scalar.activation(ex2[:, j, :], ps_s, AF.Exp, bias=nshift_t, scale=EXPSCALE)
                for fn in extras[t]:
                    fn()
                # Z/uT matmuls for the PREVIOUS iteration (exp already done)
                if prev_ex is not None:
                    tp = t - 1
                    nc.tensor.matmul(
                        ps_z, lhsT=ones_f8w, rhs=prev_ex,
                        start=(tp == 0), stop=False, perf_mode=DR,
                    )
                    for es in range(4):
                        nc.tensor.matmul(
                            ps_uT[:, es, :],
                            lhsT=obs_f8[:, 2 * tp : 2 * tp + 2, es * P : (es + 1) * P],
                            rhs=prev_ex,
                            start=(tp == 0), stop=False, perf_mode=DR,
                        )
                prev_ex = ex2
            # final iteration's Z/uT
            nc.tensor.matmul(
                ps_z, lhsT=ones_f8w, rhs=prev_ex,
                start=False, stop=True, perf_mode=DR,
            )
            for es in range(4):
                nc.tensor.matmul(
                    ps_uT[:, es, :],
                    lhsT=obs_f8[:, 30:32, es * P : (es + 1) * P],
                    rhs=prev_ex,
                    start=False, stop=True, perf_mode=DR,
                )

            # drain uT (+ self term), fp8, pre-scaled by UTSCALE
            uT = utp.tile([P, 4, bs], f8, tag="uTsb")
            for ec in range(4):
                nc.vector.scalar_tensor_tensor(
                    uT[:, ec, :],
                    in0=ps_uT[:, ec, :],
                    scalar=UTSCALE,
                    in1=w0v[:, ec, nsl],
                    op0=ALU.mult,
                    op1=ALU.add,
                )
            uts.append(uT)

            # Z -> per-token recipZ (rank-1 matmuls = on-chip transpose)
            zr = zsmall.tile([1, bs], f32, tag="zr")
            nc.vector.tensor_add(zr, ps_z[0:1, :], w0row[:, nsl])
            ps_zt = ps_sh_pool.tile([P, nch], f32, tag="sh")
            for k in range(nch):
                nc.tensor.matmul(
                    ps_zt[:, k : k + 1],
                    lhsT=zr[:, k * P : (k + 1) * P],
                    rhs=ones_f32,
                    start=True,
                    stop=True,
                )
            rz = zsmall.tile([P, nch], f32, tag="rz")
            nc.vector.reciprocal(rz, ps_zt)
            nc.vector.tensor_scalar(
                rz, in0=rz, scalar1=RZSCALE, scalar2=None, op0=ALU.mult
            )
            rzs.append(rz)
            if dbg and b == 0:
                nc.gpsimd.dma_start(dbg_t["dbg_uT0"].ap(), uT)
                nc.gpsimd.dma_start(dbg_t["dbg_zr0"].ap(), zr)
                nc.gpsimd.dma_start(dbg_t["dbg_rz0"].ap(), rz)

            if b == nblk - 1:
                for k in range(nch):
                    epi_chunk(b, k)
        if dbg:
            nc.gpsimd.dma_start(dbg_t["dbg_obs8"].ap(), obs_f8)
            nc.gpsimd.dma_start(dbg_t["dbg_obsT"].ap(), obsT)
            nc.gpsimd.dma_start(dbg_t["dbg_vT"].ap(), vT)
            nc.gpsimd.dma_start(dbg_t["dbg_vAT"].ap(), vAT)
            nc.gpsimd.dma_start(dbg_t["dbg_A"].ap(), A_sb)
            nc.gpsimd.dma_start(dbg_t["dbg_WvT"].ap(), WvT)
            nc.gpsimd.dma_start(dbg_t["dbg_w0row"].ap(), w0row)
            nc.gpsimd.dma_start(dbg_t["dbg_w0bc"].ap(), w0bc)

    nc.compile()
    return nc


def _get_nc():
    global _CACHED_NC
    if _CACHED_NC is None:
        _CACHED_NC = _build()
    return _CACHED_NC


def _in_maps(v_code, obs_code, Wq, Wk, Wv, gamma, beta):
    def f(x):
        return np.ascontiguousarray(np.asarray(x), dtype=np.float32)

    shared = {
        "obs_code": f(obs_code),
        "Wq": f(Wq),
        "Wk": f(Wk),
        "Wv": f(Wv),
        "gamma": f(gamma),
        "beta": f(beta),
    }
    return [
        {"v_code": f(v_code[c * NLOC : (c + 1) * NLOC]), **shared}
        for c in range(CORES)
    ]


def run(trace=False, **inputs):
    from concourse.bass_utils import run_bass_kernel_spmd

    nc = _get_nc()
    res = run_bass_kernel_spmd(
        nc, _in_maps(**inputs), core_ids=list(range(CORES)), trace=trace
    )
    out = np.concatenate(
        [res.results[c]["out"] for c in range(CORES)], axis=0
    ).astype(np.float32)
    return out, res


def kernel(**inputs) -> np.ndarray:
    out, _ = run(trace=False, **inputs)
    return out
